# revision 53
# baseline (speedup 1.0000x reference)
"""Trainium2 Bass kernel for nn_Attention_38182259261827.

Multi-head attention (B=4, C=512, L=2048, H=8, D=64) with pointwise-conv
QKV / output projections, ~206-212 us on 8 NeuronCores (SPMD, no
collectives).

Sharding: core c handles batch b=c//2, head-group g=c%2 (4 heads each).
Inputs are sharded AND pre-cast to bf16 host-side; each core computes
its partial output-projection sum over its 4 heads and the two partials
per batch are summed host-side (plus bias).

Per-core pipeline (all matmuls bf16, f32 PSUM accumulation):
  1. PE warmup matmuls un-throttle the HAM clock gate during input DMA
     (issued on both HWDGE queues: weights on sync, x on scalar).
  2. qk projection: qk[f, l], f = 4 heads x (q|k) packed 128/chunk.
  3. v^T projection: va[l, 4, 65] = per-head [v^T | ones]; the ones
     column makes the PV matmul emit the softmax row-sums for free.
  4. Per (i-tile 512, head) strip: S^T[j, i] = k^T q on PE (K=64);
     exp(S*scale) on ACT over j-chunk groups [3,3,3,3,2,2] (no
     max-subtract needed: S ~ N(0,1), |S|<8); PV with va stationary /
     exp moving accumulates O^T[65, i] in PSUM (row 64 = softmax
     denominators). The S^T/exp emission leads PV by two groups so ACT
     never starves at strip boundaries; leftover projection chains are
     fed into the pipeline as PE fillers with just-in-time deadlines.
     Normalize: DVE approx-reciprocal of the sums row, partition-
     broadcast via a free-dim stride-0 DMA, DVE multiply into the O^T
     sbuf layout the output projection consumes.
  5. All output projections + DMA run in the drain phase (keeping them
     out of the steady state avoids PSUM slot contention; ysb copies
     alternate DVE/ACT and output DMAs alternate the two HWDGE queues).

PSUM budget: st 2x3 banks (S^T double-buffer) + o 2x1 (O^T accumulators)
= 8 banks; the out-projection reuses those tags in the drain phase.

Numerics: bf16 matmuls everywhere, exp in f32->bf16, f32 normalize;
rel err vs the f32 reference ~5.2e-3. PSUM accumulation caveat: a
matmul with start=True clears has_written for the WHOLE bank, so only
one accumulation chain per bank may use it.
"""

import sys

if "/opt/trn_rl_repo" not in sys.path:
    sys.path.insert(0, "/opt/trn_rl_repo")

import numpy as np
import ml_dtypes

import concourse.bass as bass
import concourse.mybir as mybir
from concourse import bacc
from concourse.tile import TileContext
from concourse.bass_utils import run_bass_kernel_spmd

F32 = mybir.dt.float32
BF16 = mybir.dt.bfloat16
EXP = mybir.ActivationFunctionType.Exp

B, C, L = 4, 512, 2048
HEADS, D = 8, 64
HL = HEADS // 2          # 4 local heads per core
SCALE = D ** -0.5        # 0.125
N_CORES = 8
NKC = C // 128           # 4 contraction chunks
NLT = L // 512           # 4 l/i tiles of 512
NLC = L // 128           # 16 l/j chunks of 128
J_GROUPS = [[0, 1], [2, 3, 4], [5, 6, 7], [8, 9, 10], [11, 12, 13], [14, 15]]
N_WARMUP = 36            # PE pre-warm matmuls (HAM un-throttle)

_NC_CACHE = None


def _build_nc():
    nc = bacc.Bacc(
        "TRN2",
        target_bir_lowering=False,
        debug=False,
        enable_asserts=False,
        num_devices=N_CORES,
    )
    x_d = nc.dram_tensor("x", [C, L], BF16, kind="ExternalInput")
    wqk_d = nc.dram_tensor("wqk", [C, 512], BF16, kind="ExternalInput")
    wv_d = nc.dram_tensor("wv", [C, 256], BF16, kind="ExternalInput")
    wo_d = nc.dram_tensor("wout", [256, C], BF16, kind="ExternalInput")
    out_d = nc.dram_tensor("out", [C, L], F32, kind="ExternalOutput")

    with TileContext(nc) as tc:
        with (
            tc.tile_pool(name="sb", bufs=1) as SB,
            tc.tile_pool(name="ps", space="PSUM", bufs=1) as PS,
        ):
            # --- PE warmup: dense matmuls with no data deps so the HAM
            #     un-throttles while the input DMAs stream in.
            warm = SB.tile([128, 512], BF16, tag="warm", bufs=1, name="warm")
            nc.gpsimd.memset(warm, 0.0)
            for w in range(N_WARMUP):
                wp = PS.tile([128, 512], F32, tag="o", bufs=2, name=f"wp{w}")
                nc.tensor.matmul(wp, warm[:, 0:128], warm, start=True, stop=True)

            # --- weights: DMA + cast to bf16 ---
            wqk_bf = [
                SB.tile([128, 512], BF16, tag=f"wqk{i}", bufs=1, name=f"wqk{i}")
                for i in range(NKC)
            ]
            wv_bf = [
                SB.tile([128, 256], BF16, tag=f"wv{i}", bufs=1, name=f"wv{i}")
                for i in range(NKC)
            ]
            wo_bf = [
                SB.tile([128, 512], BF16, tag=f"wo{i}", bufs=1, name=f"wo{i}")
                for i in range(2)
            ]
            for i in range(NKC):
                nc.sync.dma_start(out=wqk_bf[i], in_=wqk_d[i * 128:(i + 1) * 128, :])
            for i in range(NKC):
                nc.sync.dma_start(out=wv_bf[i], in_=wv_d[i * 128:(i + 1) * 128, :])
            x_bf = [
                SB.tile([128, L], BF16, tag=f"x{i}", bufs=1, name=f"x{i}")
                for i in range(NKC)
            ]
            for i in range(NKC):
                nc.scalar.dma_start(out=x_bf[i], in_=x_d[i * 128:(i + 1) * 128, :])
            for i in range(2):
                nc.sync.dma_start(out=wo_bf[i], in_=wo_d[i * 128:(i + 1) * 128, :])

            # --- qk projection: qk_bf[m][f 128, l 2048], m-chunks:
            #     m=0: q_h0|q_h1, m=1: q_h2|q_h3, m=2: k_h0|k_h1, m=3: k_h2|k_h3
            qk_bf = [
                SB.tile([128, L], BF16, tag=f"qk{m}", bufs=1, name=f"qk{m}")
                for m in range(4)
            ]
            def qk_chain(m, lt, on_act=True):
                pp = PS.tile([128, 512], F32, tag="st", bufs=2, name=f"ppqk{m}_{lt}")
                for kc in range(NKC):
                    nc.tensor.matmul(
                        pp,
                        wqk_bf[kc][:, m * 128:(m + 1) * 128],
                        x_bf[kc][:, lt * 512:(lt + 1) * 512],
                        start=(kc == 0),
                        stop=(kc == NKC - 1),
                    )
                if on_act:
                    nc.scalar.copy(qk_bf[m][:, lt * 512:(lt + 1) * 512], pp)
                else:
                    nc.vector.tensor_copy(qk_bf[m][:, lt * 512:(lt + 1) * 512], pp)

            for lt in range(NLT):
                qk_chain(2, lt)

            # --- v^T projection into va[l-chunk partitions, (lc*4+h), 0:64],
            #     col 64 of each head's block stays 1.0 (softmax row-sums)
            va = SB.tile([128, 4 * NLC, 65], BF16, tag="va", bufs=1, name="va")
            nc.vector.memset(va, 1.0)

            def v_chain(lc):
                vp = PS.tile([128, 256], F32, tag="st", bufs=2, name=f"vp{lc}")
                for kc in range(NKC):
                    nc.tensor.matmul(
                        vp,
                        x_bf[kc][:, lc * 128:(lc + 1) * 128],
                        wv_bf[kc],
                        start=(kc == 0),
                        stop=(kc == NKC - 1),
                    )
                nc.vector.tensor_copy(
                    va[:, lc * 4:(lc + 1) * 4, 0:64],
                    vp.rearrange("p (h d) -> p h d", h=4),
                )

            for lc in range(6):
                v_chain(lc)
            qk_chain(0, 0)
            qk_chain(3, 0)

            # --- attention + output projection, i-tile outer ---
            OT_bf = [
                SB.tile([128, L], BF16, tag=f"otb{i}", bufs=1, name=f"otb{i}")
                for i in range(2)
            ]
            def out_chain(it, fm):
                tag = "st" if it < 2 else "o"
                yp = PS.tile([128, 512], F32, tag=tag, bufs=2, name=f"yp{it}_{fm}")
                for kc in range(2):
                    nc.tensor.matmul(
                        yp,
                        wo_bf[kc][:, fm * 128:(fm + 1) * 128],
                        OT_bf[kc][:, it * 512:(it + 1) * 512],
                        start=(kc == 0),
                        stop=(kc == 1),
                    )
                ysb = SB.tile([128, 512], F32, tag="ysb", bufs=4, name=f"ysb{it}_{fm}")
                if fm % 2 == 0:
                    nc.vector.tensor_copy(ysb, yp)
                else:
                    nc.scalar.copy(ysb, yp)
                eng = nc.sync if fm % 2 == 0 else nc.scalar
                eng.dma_start(
                    out=out_d[fm * 128:(fm + 1) * 128, it * 512:(it + 1) * 512],
                    in_=ysb,
                )

            # Flat software pipeline over (i-tile, head, j-group): emit the
            # NEXT group's S^T + exp before the CURRENT group's PV so the
            # ACT engine never waits for the PE queue to clear a strip tail.
            strips = [(it, h) for it in range(NLT) for h in range(HL)]
            o_tiles = {}

            def emit_st(it, h, grp, gi0):
                qp = 64 * (h % 2)
                q_ap = qk_bf[h // 2][qp:qp + 64, it * 512:(it + 1) * 512]
                st = PS.tile(
                    [128, 3, 512], F32, tag="st", bufs=2, name=f"st{it}_{h}_{gi0}"
                )
                for gi, jc in enumerate(grp):
                    k_ap = qk_bf[2 + h // 2][qp:qp + 64, jc * 128:(jc + 1) * 128]
                    nc.tensor.matmul(st[:, gi, :], k_ap, q_ap, start=True, stop=True)
                g = len(grp)
                ex = SB.tile(
                    [128, 3, 512], BF16, tag="exp", bufs=4, name=f"ex{it}_{h}_{gi0}"
                )
                nc.scalar.activation(
                    ex[:, 0:g, :], st[:, 0:g, :], EXP, scale=float(SCALE)
                )
                return ex

            def emit_pv(it, h, grp, ex):
                o_ps = o_tiles[(it, h)]
                for gi, jc in enumerate(grp):
                    nc.tensor.matmul(
                        o_ps,
                        va[:, jc * 4 + h, :],
                        ex[:, gi, :],
                        start=(jc == 0),
                        stop=(jc == NLC - 1),
                    )

            def emit_norm(it, h):
                o_ps = o_tiles.pop((it, h))
                rsum = SB.tile([1, 512], F32, tag="rsum", bufs=2, name=f"rsum{it}_{h}")
                nc.vector.tensor_copy(rsum, o_ps[64:65, :])
                rs = SB.tile([1, 512], F32, tag="rs", bufs=2, name=f"rs{it}_{h}")
                nc.vector.reciprocal_approx_fast(rs, rsum)
                rb = SB.tile([64, 512], F32, tag="rb", bufs=2, name=f"rb{it}_{h}")
                rs_rep = bass.AP(
                    tensor=rs.tensor,
                    offset=rs.offset,
                    ap=[[1, 1], [0, 64], [1, 512]],
                )
                nc.sync.dma_start(out=rb, in_=rs_rep)
                cp = 64 * (h % 2)
                nc.vector.tensor_mul(
                    OT_bf[h // 2][cp:cp + 64, it * 512:(it + 1) * 512],
                    o_ps[0:64, :],
                    rb,
                )

            flat = [
                (it, h, grp, gi0)
                for (it, h) in strips
                for gi0, grp in enumerate(J_GROUPS)
            ]
            # remaining projection chains, fed one per pipeline group so
            # they fill PE slack without starving the ACT engine
            filler_q = [
                ("v", 6), ("v", 7), ("v", 8), ("v", 9), ("v", 10), ("v", 11),
                ("v", 12), ("v", 13), ("v", 14), ("v", 15),
                (3, 1), (3, 2), (3, 3), (1, 0),
                (0, 1), (1, 1), (0, 2), (1, 2), (0, 3), (1, 3),
            ]
            # S^T/exp emission leads the PV consumption by two pipeline
            # steps (the two st slots hold the in-flight groups), so the ACT
            # engine's next exp is always already queued when a strip ends.
            from collections import deque
            exq = deque()  # (it, h, grp, ex) awaiting PV, depth 2

            def lead_alloc(idx):
                it, h, grp, gi0 = flat[idx]
                if gi0 == 0:
                    o_tiles[(it, h)] = PS.tile(
                        [65, 512], F32, tag="o", bufs=2, name=f"o{it}_{h}"
                    )
                exq.append((it, h, grp, emit_st(it, h, grp, gi0)))

            lead_alloc(0)
            lead_alloc(1)
            for fi in range(len(flat)):
                if fi + 2 < len(flat):
                    lead_alloc(fi + 2)
                n_pop = 2 if fi < 8 else 1
                for _ in range(n_pop):
                    if filler_q:
                        kind, a = filler_q.pop(0)
                        if kind == "v":
                            v_chain(a)
                        else:
                            qk_chain(kind, a)
                pit, ph, pgrp, pex = exq.popleft()
                emit_pv(pit, ph, pgrp, pex)
                if pgrp is J_GROUPS[-1]:
                    emit_norm(pit, ph)
            for it in range(NLT - 1):
                for fm in range(4):
                    out_chain(it, fm)
            # keep the PE HAM-warm while the last normalize chain drains
            for w in range(8):
                wp2 = PS.tile([128, 512], F32, tag="st", bufs=2, name=f"wp2_{w}")
                nc.tensor.matmul(wp2, warm[:, 0:128], warm, start=True, stop=True)
            for fm in range(4):
                out_chain(NLT - 1, fm)
    nc.compile()
    return nc


def _shard_inputs(x, w_qkv, w_out):
    """Per-core input maps (bf16), core c = (batch c//2, head-group c%2)."""
    bf = ml_dtypes.bfloat16
    in_maps = []
    for c in range(N_CORES):
        b, g = c // 2, c % 2
        cols = slice(g * 256, (g + 1) * 256)
        wqk_c = np.ascontiguousarray(
            np.concatenate(
                [w_qkv[:, 0:512][:, cols], w_qkv[:, 512:1024][:, cols]], axis=1
            ).astype(bf)
        )
        wv_c = np.ascontiguousarray(w_qkv[:, 1024:1536][:, cols].astype(bf))
        wo_c = np.ascontiguousarray(w_out[g * 256:(g + 1) * 256, :].astype(bf))
        in_maps.append(
            {
                "x": np.ascontiguousarray(x[b].astype(bf)),
                "wqk": wqk_c,
                "wv": wv_c,
                "wout": wo_c,
            }
        )
    return in_maps


def _run(x, w_qkv, w_out, b_out, trace=False, tmpdir=None):
    global _NC_CACHE
    if _NC_CACHE is None:
        _NC_CACHE = _build_nc()
    nc = _NC_CACHE
    in_maps = _shard_inputs(
        np.asarray(x, np.float32),
        np.asarray(w_qkv, np.float32),
        np.asarray(w_out, np.float32),
    )
    res = run_bass_kernel_spmd(
        nc, in_maps, core_ids=list(range(N_CORES)), trace=trace, tmpdir=tmpdir
    )
    b_out = np.asarray(b_out, np.float32)
    y = np.empty((B, C, L), np.float32)
    for b in range(B):
        y[b] = res.results[2 * b]["out"] + res.results[2 * b + 1]["out"] + b_out[:, None]
    return y, res


def kernel(x, w_qkv, w_out, b_out):
    y, _ = _run(x, w_qkv, w_out, b_out, trace=False)
    return y


if __name__ == "__main__":
    rng = np.random.default_rng(0)
    x = rng.standard_normal((B, C, L)).astype(np.float32)
    w_qkv = (rng.standard_normal((C, 3 * 512)) * C ** -0.5).astype(np.float32)
    w_out = (rng.standard_normal((512, C)) * 512 ** -0.5).astype(np.float32)
    b_out = np.zeros((C,), np.float32)
    y = kernel(x=x, w_qkv=w_qkv, w_out=w_out, b_out=b_out)
    print("ran ok", y.shape, y.dtype)


# revision 54
# speedup vs baseline: 1.0183x; 1.0183x over previous
"""Trainium2 Bass kernel for nn_Attention_38182259261827.

Multi-head attention (B=4, C=512, L=2048, H=8, D=64) with pointwise-conv
QKV / output projections, ~206-212 us on 8 NeuronCores (SPMD, no
collectives).

Sharding: core c handles batch b=c//2, head-group g=c%2 (4 heads each).
Inputs are sharded AND pre-cast to bf16 host-side; each core computes
its partial output-projection sum over its 4 heads and the two partials
per batch are summed host-side (plus bias).

Per-core pipeline (all matmuls bf16, f32 PSUM accumulation):
  1. PE warmup matmuls un-throttle the HAM clock gate during input DMA
     (issued on both HWDGE queues: weights on sync, x on scalar).
  2. qk projection: qk[f, l], f = 4 heads x (q|k) packed 128/chunk.
  3. v^T projection: va[l, 4, 65] = per-head [v^T | ones]; the ones
     column makes the PV matmul emit the softmax row-sums for free.
  4. Per (i-tile 512, head) strip: S^T[j, i] = k^T q on PE (K=64);
     exp(S*scale) on ACT over j-chunk groups [3,3,3,3,2,2] (no
     max-subtract needed: S ~ N(0,1), |S|<8); PV with va stationary /
     exp moving accumulates O^T[65, i] in PSUM (row 64 = softmax
     denominators). The S^T/exp emission leads PV by two groups so ACT
     never starves at strip boundaries; leftover projection chains are
     fed into the pipeline as PE fillers with just-in-time deadlines.
     Normalize: DVE approx-reciprocal of the sums row, partition-
     broadcast via a free-dim stride-0 DMA, DVE multiply into the O^T
     sbuf layout the output projection consumes.
  5. All output projections + DMA run in the drain phase (keeping them
     out of the steady state avoids PSUM slot contention; ysb copies
     alternate DVE/ACT and output DMAs alternate the two HWDGE queues).

PSUM budget: st 2x3 banks (S^T double-buffer) + o 2x1 (O^T accumulators)
= 8 banks; the out-projection reuses those tags in the drain phase.

Numerics: bf16 matmuls everywhere, exp in f32->bf16, f32 normalize;
rel err vs the f32 reference ~5.2e-3. PSUM accumulation caveat: a
matmul with start=True clears has_written for the WHOLE bank, so only
one accumulation chain per bank may use it.
"""

import sys

if "/opt/trn_rl_repo" not in sys.path:
    sys.path.insert(0, "/opt/trn_rl_repo")

import numpy as np
import ml_dtypes

import concourse.bass as bass
import concourse.mybir as mybir
from concourse import bacc
from concourse.tile import TileContext
from concourse.bass_utils import run_bass_kernel_spmd

F32 = mybir.dt.float32
BF16 = mybir.dt.bfloat16
EXP = mybir.ActivationFunctionType.Exp

B, C, L = 4, 512, 2048
HEADS, D = 8, 64
HL = HEADS // 2          # 4 local heads per core
SCALE = D ** -0.5        # 0.125
N_CORES = 8
NKC = C // 128           # 4 contraction chunks
NLT = L // 512           # 4 l/i tiles of 512
NLC = L // 128           # 16 l/j chunks of 128
J_GROUPS = [[0, 1], [2, 3, 4], [5, 6, 7], [8, 9, 10], [11, 12, 13], [14, 15]]
N_WARMUP = 36            # PE pre-warm matmuls (HAM un-throttle)

_NC_CACHE = None


def _build_nc():
    nc = bacc.Bacc(
        "TRN2",
        target_bir_lowering=False,
        debug=False,
        enable_asserts=False,
        num_devices=N_CORES,
    )
    x_d = nc.dram_tensor("x", [C, L], BF16, kind="ExternalInput")
    wqk_d = nc.dram_tensor("wqk", [C, 512], BF16, kind="ExternalInput")
    wv_d = nc.dram_tensor("wv", [C, 256], BF16, kind="ExternalInput")
    wo_d = nc.dram_tensor("wout", [256, C], BF16, kind="ExternalInput")
    out_d = nc.dram_tensor("out", [C, L], F32, kind="ExternalOutput")

    with TileContext(nc) as tc:
        with (
            tc.tile_pool(name="sb", bufs=1) as SB,
            tc.tile_pool(name="ps", space="PSUM", bufs=1) as PS,
        ):
            # --- PE warmup: dense matmuls with no data deps so the HAM
            #     un-throttles while the input DMAs stream in.
            warm = SB.tile([128, 512], BF16, tag="warm", bufs=1, name="warm")
            nc.gpsimd.memset(warm, 0.0)
            for w in range(N_WARMUP):
                wp = PS.tile([128, 512], F32, tag="o", bufs=2, name=f"wp{w}")
                nc.tensor.matmul(wp, warm[:, 0:128], warm, start=True, stop=True)

            # --- weights: DMA + cast to bf16 ---
            wqk_bf = [
                SB.tile([128, 512], BF16, tag=f"wqk{i}", bufs=1, name=f"wqk{i}")
                for i in range(NKC)
            ]
            wv_bf = [
                SB.tile([128, 256], BF16, tag=f"wv{i}", bufs=1, name=f"wv{i}")
                for i in range(NKC)
            ]
            wo_bf = [
                SB.tile([128, 512], BF16, tag=f"wo{i}", bufs=1, name=f"wo{i}")
                for i in range(2)
            ]
            for i in range(NKC):
                nc.sync.dma_start(out=wqk_bf[i], in_=wqk_d[i * 128:(i + 1) * 128, :])
            for i in range(NKC):
                nc.sync.dma_start(out=wv_bf[i], in_=wv_d[i * 128:(i + 1) * 128, :])
            x_bf = [
                SB.tile([128, L], BF16, tag=f"x{i}", bufs=1, name=f"x{i}")
                for i in range(NKC)
            ]
            for i in range(NKC):
                nc.scalar.dma_start(out=x_bf[i], in_=x_d[i * 128:(i + 1) * 128, :])
            for i in range(2):
                nc.sync.dma_start(out=wo_bf[i], in_=wo_d[i * 128:(i + 1) * 128, :])

            # --- qk projection: qk_bf[m][f 128, l 2048], m-chunks:
            #     m=0: q_h0|q_h1, m=1: q_h2|q_h3, m=2: k_h0|k_h1, m=3: k_h2|k_h3
            qk_bf = [
                SB.tile([128, L], BF16, tag=f"qk{m}", bufs=1, name=f"qk{m}")
                for m in range(4)
            ]
            def qk_chain(m, lt, on_act=True):
                pp = PS.tile([128, 512], F32, tag="st", bufs=2, name=f"ppqk{m}_{lt}")
                for kc in range(NKC):
                    nc.tensor.matmul(
                        pp,
                        wqk_bf[kc][:, m * 128:(m + 1) * 128],
                        x_bf[kc][:, lt * 512:(lt + 1) * 512],
                        start=(kc == 0),
                        stop=(kc == NKC - 1),
                    )
                if on_act:
                    nc.scalar.copy(qk_bf[m][:, lt * 512:(lt + 1) * 512], pp)
                else:
                    nc.vector.tensor_copy(qk_bf[m][:, lt * 512:(lt + 1) * 512], pp)

            for lt in range(NLT):
                qk_chain(2, lt)

            # --- v^T projection into va[l-chunk partitions, (lc*4+h), 0:64],
            #     col 64 of each head's block stays 1.0 (softmax row-sums)
            va = SB.tile([128, 4 * NLC, 65], BF16, tag="va", bufs=1, name="va")
            nc.vector.memset(va, 1.0)

            def v_chain(lc):
                vp = PS.tile([128, 256], F32, tag="st", bufs=2, name=f"vp{lc}")
                for kc in range(NKC):
                    nc.tensor.matmul(
                        vp,
                        x_bf[kc][:, lc * 128:(lc + 1) * 128],
                        wv_bf[kc],
                        start=(kc == 0),
                        stop=(kc == NKC - 1),
                    )
                nc.vector.tensor_copy(
                    va[:, lc * 4:(lc + 1) * 4, 0:64],
                    vp.rearrange("p (h d) -> p h d", h=4),
                )

            for lc in range(6):
                v_chain(lc)
            qk_chain(0, 0)
            qk_chain(3, 0)

            # --- attention + output projection, i-tile outer ---
            OT_bf = [
                SB.tile([128, L], BF16, tag=f"otb{i}", bufs=1, name=f"otb{i}")
                for i in range(2)
            ]
            def out_chain(it, fm):
                tag = "st" if it < 2 else "o"
                yp = PS.tile([128, 512], F32, tag=tag, bufs=2, name=f"yp{it}_{fm}")
                for kc in range(2):
                    nc.tensor.matmul(
                        yp,
                        wo_bf[kc][:, fm * 128:(fm + 1) * 128],
                        OT_bf[kc][:, it * 512:(it + 1) * 512],
                        start=(kc == 0),
                        stop=(kc == 1),
                    )
                ysb = SB.tile([128, 512], F32, tag="ysb", bufs=4, name=f"ysb{it}_{fm}")
                if fm % 2 == 0:
                    nc.vector.tensor_copy(ysb, yp)
                else:
                    nc.scalar.copy(ysb, yp)
                eng = nc.sync if fm % 2 == 0 else nc.scalar
                eng.dma_start(
                    out=out_d[fm * 128:(fm + 1) * 128, it * 512:(it + 1) * 512],
                    in_=ysb,
                )

            # Flat software pipeline over (i-tile, head, j-group): emit the
            # NEXT group's S^T + exp before the CURRENT group's PV so the
            # ACT engine never waits for the PE queue to clear a strip tail.
            strips = [(it, h) for it in range(NLT) for h in range(HL)]
            o_tiles = {}

            def emit_st(it, h, grp, gi0):
                qp = 64 * (h % 2)
                q_ap = qk_bf[h // 2][qp:qp + 64, it * 512:(it + 1) * 512]
                st = PS.tile(
                    [128, 3, 512], F32, tag="st", bufs=2, name=f"st{it}_{h}_{gi0}"
                )
                for gi, jc in enumerate(grp):
                    k_ap = qk_bf[2 + h // 2][qp:qp + 64, jc * 128:(jc + 1) * 128]
                    nc.tensor.matmul(st[:, gi, :], k_ap, q_ap, start=True, stop=True)
                g = len(grp)
                ex = SB.tile(
                    [128, 3, 512], BF16, tag="exp", bufs=4, name=f"ex{it}_{h}_{gi0}"
                )
                nc.scalar.activation(
                    ex[:, 0:g, :], st[:, 0:g, :], EXP, scale=float(SCALE)
                )
                return ex

            def emit_pv(it, h, grp, ex):
                o_ps = o_tiles[(it, h)]
                for gi, jc in enumerate(grp):
                    nc.tensor.matmul(
                        o_ps,
                        va[:, jc * 4 + h, :],
                        ex[:, gi, :],
                        start=(jc == 0),
                        stop=(jc == NLC - 1),
                    )

            def emit_norm(it, h):
                o_ps = o_tiles.pop((it, h))
                rsum = SB.tile([1, 512], F32, tag="rsum", bufs=2, name=f"rsum{it}_{h}")
                nc.vector.tensor_copy(rsum, o_ps[64:65, :])
                rs = SB.tile([1, 512], F32, tag="rs", bufs=2, name=f"rs{it}_{h}")
                nc.vector.reciprocal_approx_fast(rs, rsum)
                rb = SB.tile([64, 512], F32, tag="rb", bufs=2, name=f"rb{it}_{h}")
                rs_rep = bass.AP(
                    tensor=rs.tensor,
                    offset=rs.offset,
                    ap=[[1, 1], [0, 64], [1, 512]],
                )
                nc.sync.dma_start(out=rb, in_=rs_rep)
                cp = 64 * (h % 2)
                nc.vector.tensor_mul(
                    OT_bf[h // 2][cp:cp + 64, it * 512:(it + 1) * 512],
                    o_ps[0:64, :],
                    rb,
                )

            flat = [
                (it, h, grp, gi0)
                for (it, h) in strips
                for gi0, grp in enumerate(J_GROUPS)
            ]
            # remaining projection chains, fed one per pipeline group so
            # they fill PE slack without starving the ACT engine
            filler_q = [
                ("v", 6), ("v", 7), ("v", 8), ("v", 9), ("v", 10), ("v", 11),
                ("v", 12), ("v", 13), ("v", 14), ("v", 15),
                (3, 1), (3, 2), (3, 3), (1, 0),
                (0, 1), (1, 1), (0, 2), (1, 2), (0, 3), (1, 3),
            ]
            # S^T/exp emission leads the PV consumption by two pipeline
            # steps (the two st slots hold the in-flight groups), so the ACT
            # engine's next exp is always already queued when a strip ends.
            from collections import deque
            exq = deque()  # (it, h, grp, ex) awaiting PV, depth 2

            def lead_alloc(idx):
                it, h, grp, gi0 = flat[idx]
                if gi0 == 0:
                    o_tiles[(it, h)] = PS.tile(
                        [65, 512], F32, tag="o", bufs=2, name=f"o{it}_{h}"
                    )
                exq.append((it, h, grp, emit_st(it, h, grp, gi0)))

            lead_alloc(0)
            lead_alloc(1)
            for fi in range(len(flat)):
                if fi + 2 < len(flat):
                    lead_alloc(fi + 2)
                n_pop = 2 if fi < 8 else 1
                for _ in range(n_pop):
                    if filler_q:
                        kind, a = filler_q.pop(0)
                        if kind == "v":
                            v_chain(a)
                        else:
                            qk_chain(kind, a)
                pit, ph, pgrp, pex = exq.popleft()
                emit_pv(pit, ph, pgrp, pex)
                if pgrp is J_GROUPS[-1]:
                    emit_norm(pit, ph)
            for it in range(NLT):
                for fm in range(4):
                    out_chain(it, fm)
    nc.compile()
    return nc


def _shard_inputs(x, w_qkv, w_out):
    """Per-core input maps (bf16), core c = (batch c//2, head-group c%2)."""
    bf = ml_dtypes.bfloat16
    in_maps = []
    for c in range(N_CORES):
        b, g = c // 2, c % 2
        cols = slice(g * 256, (g + 1) * 256)
        wqk_c = np.ascontiguousarray(
            np.concatenate(
                [w_qkv[:, 0:512][:, cols], w_qkv[:, 512:1024][:, cols]], axis=1
            ).astype(bf)
        )
        wv_c = np.ascontiguousarray(w_qkv[:, 1024:1536][:, cols].astype(bf))
        wo_c = np.ascontiguousarray(w_out[g * 256:(g + 1) * 256, :].astype(bf))
        in_maps.append(
            {
                "x": np.ascontiguousarray(x[b].astype(bf)),
                "wqk": wqk_c,
                "wv": wv_c,
                "wout": wo_c,
            }
        )
    return in_maps


def _run(x, w_qkv, w_out, b_out, trace=False, tmpdir=None):
    global _NC_CACHE
    if _NC_CACHE is None:
        _NC_CACHE = _build_nc()
    nc = _NC_CACHE
    in_maps = _shard_inputs(
        np.asarray(x, np.float32),
        np.asarray(w_qkv, np.float32),
        np.asarray(w_out, np.float32),
    )
    res = run_bass_kernel_spmd(
        nc, in_maps, core_ids=list(range(N_CORES)), trace=trace, tmpdir=tmpdir
    )
    b_out = np.asarray(b_out, np.float32)
    y = np.empty((B, C, L), np.float32)
    for b in range(B):
        y[b] = res.results[2 * b]["out"] + res.results[2 * b + 1]["out"] + b_out[:, None]
    return y, res


def kernel(x, w_qkv, w_out, b_out):
    y, _ = _run(x, w_qkv, w_out, b_out, trace=False)
    return y


if __name__ == "__main__":
    rng = np.random.default_rng(0)
    x = rng.standard_normal((B, C, L)).astype(np.float32)
    w_qkv = (rng.standard_normal((C, 3 * 512)) * C ** -0.5).astype(np.float32)
    w_out = (rng.standard_normal((512, C)) * 512 ** -0.5).astype(np.float32)
    b_out = np.zeros((C,), np.float32)
    y = kernel(x=x, w_qkv=w_qkv, w_out=w_out, b_out=b_out)
    print("ran ok", y.shape, y.dtype)
